# revision 9
# baseline (speedup 1.0000x reference)
"""Trainium2 Bass kernel for nn_CrossAttention (B=4, LQ=4096, S=4096, D=512).

Sharding: data-parallel over (batch, query-half): core = 2*b + half.

Algebraic folds (all exact in fp32):
  scores = (query @ wq + bq) @ wk^T @ tgt^T + const(q)   [bk cancels in softmax]
         = query @ WQK + gamma, applied against tgt^T    [WQK = wq @ wk^T]
  out    = softmax(scores) @ tgt @ (wv @ wo) + (bv @ wo + bo)
         = (w @ tgt)/rowsum @ WVO + b_out                [WVO = wv @ wo]
This removes the K/V/O projections entirely: no per-core redundant K/V work,
and the only big matmuls left are the two attention GEMMs at the PE roofline.

Precision: fp16 operands everywhere (same 1 cycle/row PE speed as bf16,
4x less rounding error). The exp is computed as exp(score - 17) so the
unnormalized weights fit fp16 range; the offset cancels in normalization.
ctx accumulators are cast to bf16 (their dynamic range exceeds fp16).
Emulated end-to-end error: 2.8e-3 relmax (gate 2e-2).
"""

import numpy as np

B, LQ, S = 4, 4096, 4096
D = 512
P = 128
LQH = LQ // 2    # 2048 query rows per core
DC = D // P      # 4 feature chunks
SC = S // P      # 32 s-chunks
IB = 512         # block width
NB = LQH // IB   # 4 query blocks
KB = S // IB     # 8 kv blocks
C_OFF = 17.0     # exp offset; cancels in softmax normalization

_CACHED = {}


def _build_program():
    import concourse.bass as bass
    import concourse.mybir as mybir
    import concourse.tile as tile
    from concourse import bacc
    from concourse.masks import make_identity
    from contextlib import ExitStack

    f32 = mybir.dt.float32
    fp16 = mybir.dt.float16
    bf16 = mybir.dt.bfloat16
    AF = mybir.ActivationFunctionType
    OP = mybir.AluOpType

    nc = bacc.Bacc("TRN2", target_bir_lowering=False, debug=False, num_devices=8)

    query = nc.dram_tensor("query", [LQH, D], f32, kind="ExternalInput").ap()
    target = nc.dram_tensor("target", [S, D], f32, kind="ExternalInput").ap()
    w_dram = {}
    for nm in ("wq", "wk", "wv", "wo"):
        w_dram[nm] = nc.dram_tensor(nm, [D, D], f32, kind="ExternalInput").ap()
    b_dram = {}
    for nm in ("bq", "bv", "bo"):
        b_dram[nm] = nc.dram_tensor(nm, [D], f32, kind="ExternalInput").ap()
    out_dram = nc.dram_tensor("out", [LQH, D], f32, kind="ExternalOutput").ap()

    with tile.TileContext(nc) as tc, ExitStack() as ctx:
        const = ctx.enter_context(tc.tile_pool(name="const", bufs=1))
        big = ctx.enter_context(tc.tile_pool(name="big", bufs=1))
        wst = ctx.enter_context(tc.tile_pool(name="wst", bufs=6))
        ld = ctx.enter_context(tc.tile_pool(name="ld", bufs=6))
        qld = ctx.enter_context(tc.tile_pool(name="qld", bufs=4))
        cst = ctx.enter_context(tc.tile_pool(name="cst", bufs=2))
        ptp = ctx.enter_context(tc.tile_pool(name="ptp", bufs=6))
        ctxp = ctx.enter_context(tc.tile_pool(name="ctxp", bufs=2))
        outp = ctx.enter_context(tc.tile_pool(name="outp", bufs=2))
        smallp = ctx.enter_context(tc.tile_pool(name="smallp", bufs=2))
        ps_ctx = ctx.enter_context(tc.tile_pool(name="ps_ctx", bufs=4, space="PSUM"))
        ps_m = ctx.enter_context(tc.tile_pool(name="ps_m", bufs=4, space="PSUM"))

        # ---- tiny constants ----
        ident_h = const.tile([P, P], fp16, tag="ident_h", name="ident_h")
        make_identity(nc, ident_h)
        ident_f = const.tile([P, P], f32, tag="ident_f", name="ident_f")
        make_identity(nc, ident_f)
        ones_col = const.tile([P, 1], f32, tag="ones_col", name="ones_col")
        nc.vector.memset(ones_col, 1.0)
        ones_row_h = const.tile([1, P], fp16, tag="ones_row", name="ones_row")
        nc.vector.memset(ones_row_h, 1.0)
        rstage = const.tile([P, IB], f32, tag="rstage", name="rstage")
        nc.vector.memset(rstage, 0.0)
        negc_col = const.tile([P, 1], f32, tag="negc", name="negc_col")
        nc.vector.memset(negc_col, -C_OFF)

        b_col_h = {}
        for nm in ("bq", "bv"):
            bc = const.tile([P, DC], f32, tag=f"c_{nm}", name=f"{nm}_c")
            nc.gpsimd.dma_start(out=bc, in_=b_dram[nm].rearrange("(c p) -> p c", p=P))
            bh = const.tile([P, DC], fp16, tag=f"h_{nm}", name=f"{nm}_h")
            nc.vector.tensor_copy(out=bh, in_=bc)
            b_col_h[nm] = bh
        bo_row = const.tile([1, D], f32, tag="bo_row", name="bo_row")
        nc.gpsimd.dma_start(out=bo_row,
                            in_=b_dram["bo"].rearrange("(a n) -> a n", a=1))

        # ---- weight staging / transposes ----
        def stage_weight(nm):
            # chunked DMA + cast so downstream transposes start incrementally
            wh = const.tile([P, DC, D], fp16, tag=f"wh_{nm}", name=f"{nm}_h")
            for dc in range(DC):
                wf = wst.tile([P, D], f32, tag="w_stage", name=f"{nm}_f{dc}")
                nc.sync.dma_start(wf, w_dram[nm][dc * P:(dc + 1) * P, :])
                nc.vector.tensor_copy(out=wh[:, dc, :], in_=wf)
            return wh

        def transpose_into(dst, src_h, tag):
            # dst[p, j, c*128+r] = src[c*128+r, j*128+p]
            for c in range(DC):
                psv = ps_m.tile([P, D], fp16, tag="ps_m", name=f"T{tag}_{c}")
                for j in range(DC):
                    nc.tensor.transpose(psv[:, j * P:(j + 1) * P],
                                        src_h[:, c, j * P:(j + 1) * P], ident_h)
                nc.scalar.activation(dst[:, :, c * P:(c + 1) * P],
                                     psv.rearrange("p (c q) -> p c q", c=DC),
                                     AF.Copy)

        def row_to_col(row_ps, dst_col, tag):
            # [1, D] psum row -> [P, DC] per-partition column via PE transpose
            nc.vector.tensor_copy(out=rstage[0:1, :], in_=row_ps)
            rt = ps_m.tile([P, IB], f32, tag="ps_m", name=f"rt_{tag}")
            for c in range(DC):
                nc.tensor.transpose(rt[:, c * P:(c + 1) * P],
                                    rstage[:, c * P:(c + 1) * P], ident_f)
            nc.scalar.activation(dst_col,
                                 rt.rearrange("p (c q) -> p c q", c=DC)[:, :, 0],
                                 AF.Copy)

        wq_h = stage_weight("wq")
        wqT = const.tile([P, DC, D], fp16, tag="wqT", name="wqT")
        transpose_into(wqT, wq_h, "wq")
        wk_h = stage_weight("wk")
        wkT = const.tile([P, DC, D], fp16, tag="wkT", name="wkT")
        transpose_into(wkT, wk_h, "wk")

        # WQK[din, e] = sum_d wq[din, d] * wk[e, d]
        WQK = const.tile([P, DC, D], fp16, tag="WQK", name="WQK")
        for dinc in range(DC):
            ps = ps_m.tile([P, D], f32, tag="ps_m", name=f"wqk_{dinc}")
            for dc in range(DC):
                nc.tensor.matmul(ps, wqT[:, dc, dinc * P:(dinc + 1) * P],
                                 wkT[:, dc, :], start=(dc == 0), stop=(dc == DC - 1))
            nc.scalar.activation(WQK[:, dinc, :], ps, AF.Copy)

        # gamma[e] = sum_d bq[d] * wk[e, d]  (as per-partition column chunks)
        g_ps = ps_m.tile([1, D], f32, tag="ps_m", name="g_ps")
        for dc in range(DC):
            nc.tensor.matmul(g_ps, b_col_h["bq"][:, dc:dc + 1], wkT[:, dc, :],
                             start=(dc == 0), stop=(dc == DC - 1))
        gamma_col = const.tile([P, DC], f32, tag="gamma", name="gamma_col")
        row_to_col(g_ps, gamma_col, "g")

        # ---- query side ----
        qpT = [big.tile([P, DC, IB], fp16, tag=f"qpT{i}", name=f"qpT{i}")
               for i in range(NB)]
        qstage = {}

        def q_dma(ibk):
            tiles = []
            for cc in range(DC):
                t = qld.tile([P, D], f32, tag="qld", name=f"q_{ibk}_{cc}")
                nc.sync.dma_start(t, query[ibk * IB + cc * P: ibk * IB + (cc + 1) * P, :])
                tiles.append(t)
            qstage[ibk] = tiles

        def q_prep(ibk):
            qinT = smallp.tile([P, DC, IB], fp16, tag="qinT", name=f"qinT{ibk}")
            for cc in range(DC):
                qc = cst.tile([P, D], fp16, tag="qcast", name=f"qc_{ibk}_{cc}")
                nc.vector.tensor_copy(out=qc, in_=qstage[ibk][cc])
                psv = ps_m.tile([P, D], fp16, tag="ps_m", name=f"qT_{ibk}_{cc}")
                for j in range(DC):
                    nc.tensor.transpose(psv[:, j * P:(j + 1) * P],
                                        qc[:, j * P:(j + 1) * P], ident_h)
                nc.scalar.activation(qinT[:, :, cc * P:(cc + 1) * P],
                                     psv.rearrange("p (c q) -> p c q", c=DC),
                                     AF.Copy)
            for ec in range(DC):
                ps = ps_m.tile([P, IB], f32, tag="ps_m", name=f"qp_{ibk}_{ec}")
                for dinc in range(DC):
                    nc.tensor.matmul(ps, WQK[:, dinc, ec * P:(ec + 1) * P],
                                     qinT[:, dinc, :],
                                     start=(dinc == 0), stop=(dinc == DC - 1))
                nc.scalar.activation(qpT[ibk][:, ec, :], ps, AF.Identity,
                                     bias=gamma_col[:, ec:ec + 1])

        q_dma(0)
        q_prep(0)

        # ---- target tiles (filled just-in-time during ib 0) ----
        tgtT = [big.tile([P, DC, IB], fp16, tag=f"tgtT{i}", name=f"tgtT{i}")
                for i in range(KB)]
        tgt_h = [big.tile([P, D], fp16, tag=f"tgh{i}", name=f"tgh{i}")
                 for i in range(SC)]

        def t_dma(g):
            tiles = []
            for cc in range(DC):
                t = ld.tile([P, D], f32, tag="ld", name=f"t_{g}_{cc}")
                nc.sync.dma_start(t, target[g * IB + cc * P: g * IB + (cc + 1) * P, :])
                tiles.append(t)
            return tiles

        tstage = {0: t_dma(0)}

        def t_prep(g):
            for cc in range(DC):
                sccc = g * DC + cc
                nc.vector.tensor_copy(out=tgt_h[sccc], in_=tstage[g][cc])
                psv = ps_m.tile([P, D], fp16, tag="ps_m", name=f"tT_{g}_{cc}")
                for j in range(DC):
                    nc.tensor.transpose(psv[:, j * P:(j + 1) * P],
                                        tgt_h[sccc][:, j * P:(j + 1) * P], ident_h)
                nc.scalar.activation(tgtT[g][:, :, cc * P:(cc + 1) * P],
                                     psv.rearrange("p (c q) -> p c q", c=DC),
                                     AF.Copy)
            del tstage[g]

        # remaining DMAs in consumption order on the sync queue
        wv_h = stage_weight("wv")
        wo_h = stage_weight("wo")
        for g in range(1, 3):
            tstage[g] = t_dma(g)
        q_dma(1)
        for g in range(3, 5):
            tstage[g] = t_dma(g)
        q_dma(2)
        for g in range(5, KB):
            tstage[g] = t_dma(g)
        q_dma(3)

        WVO = const.tile([P, DC, D], bf16, tag="WVO", name="WVO")
        b_out_rep = const.tile([P, D], f32, tag="b_out", name="b_out_rep")

        def vo_prep():
            wvT = const.tile([P, DC, D], fp16, tag="wvT", name="wvT")
            transpose_into(wvT, wv_h, "wv")
            for ec in range(DC):
                ps = ps_m.tile([P, D], f32, tag="ps_m", name=f"wvo_{ec}")
                for dc in range(DC):
                    nc.tensor.matmul(ps, wvT[:, dc, ec * P:(ec + 1) * P],
                                     wo_h[:, dc, :], start=(dc == 0), stop=(dc == DC - 1))
                nc.scalar.activation(WVO[:, ec, :], ps, AF.Copy)
            bp = ps_m.tile([1, D], f32, tag="ps_m", name="bvo_ps")
            for dc in range(DC):
                nc.tensor.matmul(bp, b_col_h["bv"][:, dc:dc + 1], wo_h[:, dc, :],
                                 start=(dc == 0), stop=(dc == DC - 1))
            br = const.tile([1, D], f32, tag="b_row", name="b_out_row")
            nc.vector.tensor_tensor(br, bp, bo_row, OP.add)
            brh = const.tile([1, D], fp16, tag="b_rowh", name="b_out_row_h")
            nc.vector.tensor_copy(out=brh, in_=br)
            bp2 = ps_m.tile([P, D], f32, tag="ps_m", name="brep_ps")
            nc.tensor.matmul(bp2, ones_row_h, brh, start=True, stop=True)
            nc.scalar.activation(b_out_rep, bp2, AF.Copy)

        # ---- attention ----
        for ib in range(NB):
            ctx_ps = [ps_ctx.tile([P, IB], f32, tag="ps_ctx", name=f"ctx_{ib}_{d}")
                      for d in range(DC)]
            acc = cst.tile([P, IB], f32, tag="acc", name=f"acc_{ib}")

            for scc in range(SC):
                g, sl = divmod(scc, IB // P)
                if ib == 0 and sl == 0:
                    t_prep(g)
                pt_ps = ps_m.tile([P, IB], f32, tag="ps_m", name=f"pt_{ib}_{scc}")
                for ec in range(DC):
                    nc.tensor.matmul(pt_ps, tgtT[g][:, ec, sl * P:(sl + 1) * P],
                                     qpT[ib][:, ec, :],
                                     start=(ec == 0), stop=(ec == DC - 1))
                pt_exp = ptp.tile([P, IB], fp16, tag="pt_exp", name=f"pte_{ib}_{scc}")
                nc.scalar.activation(pt_exp, pt_ps, AF.Exp, bias=negc_col)
                if scc == 0:
                    nc.vector.tensor_copy(out=acc, in_=pt_exp)
                else:
                    nc.vector.tensor_tensor(acc, acc, pt_exp, OP.add)
                for ec in range(DC):
                    nc.tensor.matmul(ctx_ps[ec], tgt_h[scc][:, ec * P:(ec + 1) * P],
                                     pt_exp, start=(scc == 0), stop=(scc == SC - 1))
                if ib == 0 and scc == 6:
                    vo_prep()
                if ib < NB - 1 and scc == 20:
                    q_prep(ib + 1)

            # softmax denominators
            rs_ps = ps_m.tile([1, IB], f32, tag="ps_m", name=f"rs_{ib}")
            nc.tensor.matmul(rs_ps, ones_col, acc, start=True, stop=True)
            rsum_col = cst.tile([P, DC], f32, tag="rsc", name=f"rsc_{ib}")
            row_to_col(rs_ps, rsum_col, f"rs{ib}")
            rc_col = cst.tile([P, DC], f32, tag="rcc", name=f"rc_{ib}")
            nc.vector.reciprocal(rc_col, rsum_col)

            ctxT = ctxp.tile([P, DC, IB], bf16, tag="ctxT", name=f"ctxT_{ib}")
            for ec in range(DC):
                nc.scalar.activation(ctxT[:, ec, :], ctx_ps[ec], AF.Copy)

            for qc in range(DC):
                op_ps = ps_m.tile([P, D], f32, tag="ps_m", name=f"op_{ib}_{qc}")
                for ec in range(DC):
                    nc.tensor.matmul(op_ps, ctxT[:, ec, qc * P:(qc + 1) * P],
                                     WVO[:, ec, :], start=(ec == 0), stop=(ec == DC - 1))
                ot_s = outp.tile([P, D], f32, tag="out_s", name=f"ots_{ib}_{qc}")
                nc.scalar.activation(ot_s, op_ps, AF.Copy,
                                     scale=rc_col[:, qc:qc + 1])
                ot = outp.tile([P, D], f32, tag="out_t", name=f"ot_{ib}_{qc}")
                nc.vector.tensor_tensor(ot, ot_s, b_out_rep, OP.add)
                nc.gpsimd.dma_start(
                    out_dram[ib * IB + qc * P: ib * IB + (qc + 1) * P, :], ot)

    nc.compile()
    return nc


def _get_nc():
    if "nc" not in _CACHED:
        _CACHED["nc"] = _build_program()
    return _CACHED["nc"]


def _make_in_maps(query, target, wq, bq, wk, bk, wv, bv, wo, bo):
    query = np.asarray(query, dtype=np.float32)
    target = np.asarray(target, dtype=np.float32)
    consts = {
        "wq": np.asarray(wq, np.float32), "bq": np.asarray(bq, np.float32),
        "wk": np.asarray(wk, np.float32),
        "wv": np.asarray(wv, np.float32), "bv": np.asarray(bv, np.float32),
        "wo": np.asarray(wo, np.float32), "bo": np.asarray(bo, np.float32),
    }
    in_maps = []
    for core in range(8):
        b, h = divmod(core, 2)
        in_maps.append({
            "query": np.ascontiguousarray(query[b, h * LQH:(h + 1) * LQH]),
            # faithful to the torch reshape: raw reinterpret of [512, 4096]
            "target": np.ascontiguousarray(target[b]).reshape(S, D),
            **consts,
        })
    return in_maps


def kernel(query, target, wq, bq, wk, bk, wv, bv, wo, bo):
    from concourse import bass_utils
    nc = _get_nc()
    in_maps = _make_in_maps(query, target, wq, bq, wk, bk, wv, bv, wo, bo)
    res = bass_utils.run_bass_kernel_spmd(nc, in_maps, core_ids=list(range(8)))
    out = np.empty((B, LQ, D), np.float32)
    for core in range(8):
        b, h = divmod(core, 2)
        out[b, h * LQH:(h + 1) * LQH] = res.results[core]["out"]
    return out


# revision 10
# speedup vs baseline: 1.0048x; 1.0048x over previous
"""Trainium2 Bass kernel for nn_CrossAttention (B=4, LQ=4096, S=4096, D=512).

Sharding: data-parallel over (batch, query-half): core = 2*b + half.

Algebraic folds (all exact in fp32):
  scores = (query @ wq + bq) @ wk^T @ tgt^T + const(q)   [bk cancels in softmax]
         = query @ WQK + gamma, applied against tgt^T    [WQK = wq @ wk^T]
  out    = softmax(scores) @ tgt @ (wv @ wo) + (bv @ wo + bo)
         = (w @ tgt)/rowsum @ WVO + b_out                [WVO = wv @ wo]
This removes the K/V/O projections entirely: no per-core redundant K/V work,
and the only big matmuls left are the two attention GEMMs at the PE roofline.

Precision: fp16 operands everywhere (same 1 cycle/row PE speed as bf16,
4x less rounding error). The exp is computed as exp(score - 17) so the
unnormalized weights fit fp16 range; the offset cancels in normalization.
ctx accumulators are cast to bf16 (their dynamic range exceeds fp16).
Emulated end-to-end error: 2.8e-3 relmax (gate 2e-2).
"""

import numpy as np

B, LQ, S = 4, 4096, 4096
D = 512
P = 128
LQH = LQ // 2    # 2048 query rows per core
DC = D // P      # 4 feature chunks
SC = S // P      # 32 s-chunks
IB = 512         # block width
NB = LQH // IB   # 4 query blocks
KB = S // IB     # 8 kv blocks
C_OFF = 17.0     # exp offset; cancels in softmax normalization

_CACHED = {}


def _build_program():
    import concourse.bass as bass
    import concourse.mybir as mybir
    import concourse.tile as tile
    from concourse import bacc
    from concourse.masks import make_identity
    from contextlib import ExitStack

    f32 = mybir.dt.float32
    fp16 = mybir.dt.float16
    bf16 = mybir.dt.bfloat16
    AF = mybir.ActivationFunctionType
    OP = mybir.AluOpType

    nc = bacc.Bacc("TRN2", target_bir_lowering=False, debug=False, num_devices=8)

    query = nc.dram_tensor("query", [LQH, D], f32, kind="ExternalInput").ap()
    target = nc.dram_tensor("target", [S, D], f32, kind="ExternalInput").ap()
    w_dram = {}
    for nm in ("wq", "wk", "wv", "wo"):
        w_dram[nm] = nc.dram_tensor(nm, [D, D], f32, kind="ExternalInput").ap()
    b_dram = {}
    for nm in ("bq", "bv", "bo"):
        b_dram[nm] = nc.dram_tensor(nm, [D], f32, kind="ExternalInput").ap()
    out_dram = nc.dram_tensor("out", [LQH, D], f32, kind="ExternalOutput").ap()

    with tile.TileContext(nc) as tc, ExitStack() as ctx:
        const = ctx.enter_context(tc.tile_pool(name="const", bufs=1))
        big = ctx.enter_context(tc.tile_pool(name="big", bufs=1))
        wst = ctx.enter_context(tc.tile_pool(name="wst", bufs=2))
        ld = ctx.enter_context(tc.tile_pool(name="ld", bufs=6))
        qld = ctx.enter_context(tc.tile_pool(name="qld", bufs=4))
        cst = ctx.enter_context(tc.tile_pool(name="cst", bufs=2))
        ptp = ctx.enter_context(tc.tile_pool(name="ptp", bufs=4))
        ctxp = ctx.enter_context(tc.tile_pool(name="ctxp", bufs=2))
        outp = ctx.enter_context(tc.tile_pool(name="outp", bufs=2))
        smallp = ctx.enter_context(tc.tile_pool(name="smallp", bufs=2))
        ps_ctx = ctx.enter_context(tc.tile_pool(name="ps_ctx", bufs=4, space="PSUM"))
        ps_m = ctx.enter_context(tc.tile_pool(name="ps_m", bufs=4, space="PSUM"))

        # ---- tiny constants ----
        ident_h = const.tile([P, P], fp16, tag="ident_h", name="ident_h")
        make_identity(nc, ident_h)
        ident_f = const.tile([P, P], f32, tag="ident_f", name="ident_f")
        make_identity(nc, ident_f)
        ones_col = const.tile([P, 1], f32, tag="ones_col", name="ones_col")
        nc.vector.memset(ones_col, 1.0)
        ones_row_h = const.tile([1, P], fp16, tag="ones_row", name="ones_row")
        nc.vector.memset(ones_row_h, 1.0)
        rstage = const.tile([P, IB], f32, tag="rstage", name="rstage")
        nc.vector.memset(rstage, 0.0)
        negc_col = const.tile([P, 1], f32, tag="negc", name="negc_col")
        nc.vector.memset(negc_col, -C_OFF)

        b_col_h = {}
        for nm in ("bq", "bv"):
            bc = const.tile([P, DC], f32, tag=f"c_{nm}", name=f"{nm}_c")
            nc.gpsimd.dma_start(out=bc, in_=b_dram[nm].rearrange("(c p) -> p c", p=P))
            bh = const.tile([P, DC], fp16, tag=f"h_{nm}", name=f"{nm}_h")
            nc.vector.tensor_copy(out=bh, in_=bc)
            b_col_h[nm] = bh
        bo_row = const.tile([1, D], f32, tag="bo_row", name="bo_row")
        nc.gpsimd.dma_start(out=bo_row,
                            in_=b_dram["bo"].rearrange("(a n) -> a n", a=1))

        # ---- weight staging / transposes ----
        def stage_weight(nm):
            wf = wst.tile([P, DC, D], f32, tag="w_stage", name=f"{nm}_f")
            nc.sync.dma_start(wf, w_dram[nm].rearrange("(c p) n -> p c n", p=P))
            wh = const.tile([P, DC, D], fp16, tag=f"wh_{nm}", name=f"{nm}_h")
            nc.vector.tensor_copy(out=wh, in_=wf)
            return wh

        def transpose_into(dst, src_h, tag):
            # dst[p, j, c*128+r] = src[c*128+r, j*128+p]
            for c in range(DC):
                psv = ps_m.tile([P, D], fp16, tag="ps_m", name=f"T{tag}_{c}")
                for j in range(DC):
                    nc.tensor.transpose(psv[:, j * P:(j + 1) * P],
                                        src_h[:, c, j * P:(j + 1) * P], ident_h)
                nc.scalar.activation(dst[:, :, c * P:(c + 1) * P],
                                     psv.rearrange("p (c q) -> p c q", c=DC),
                                     AF.Copy)

        def row_to_col(row_ps, dst_col, tag):
            # [1, D] psum row -> [P, DC] per-partition column via PE transpose
            nc.vector.tensor_copy(out=rstage[0:1, :], in_=row_ps)
            rt = ps_m.tile([P, IB], f32, tag="ps_m", name=f"rt_{tag}")
            for c in range(DC):
                nc.tensor.transpose(rt[:, c * P:(c + 1) * P],
                                    rstage[:, c * P:(c + 1) * P], ident_f)
            nc.scalar.activation(dst_col,
                                 rt.rearrange("p (c q) -> p c q", c=DC)[:, :, 0],
                                 AF.Copy)

        wq_h = stage_weight("wq")
        wqT = const.tile([P, DC, D], fp16, tag="wqT", name="wqT")
        transpose_into(wqT, wq_h, "wq")
        wk_h = stage_weight("wk")
        wkT = const.tile([P, DC, D], fp16, tag="wkT", name="wkT")
        transpose_into(wkT, wk_h, "wk")

        # WQK[din, e] = sum_d wq[din, d] * wk[e, d]
        WQK = const.tile([P, DC, D], fp16, tag="WQK", name="WQK")
        for dinc in range(DC):
            ps = ps_m.tile([P, D], f32, tag="ps_m", name=f"wqk_{dinc}")
            for dc in range(DC):
                nc.tensor.matmul(ps, wqT[:, dc, dinc * P:(dinc + 1) * P],
                                 wkT[:, dc, :], start=(dc == 0), stop=(dc == DC - 1))
            nc.scalar.activation(WQK[:, dinc, :], ps, AF.Copy)

        # gamma[e] = sum_d bq[d] * wk[e, d]  (as per-partition column chunks)
        g_ps = ps_m.tile([1, D], f32, tag="ps_m", name="g_ps")
        for dc in range(DC):
            nc.tensor.matmul(g_ps, b_col_h["bq"][:, dc:dc + 1], wkT[:, dc, :],
                             start=(dc == 0), stop=(dc == DC - 1))
        gamma_col = const.tile([P, DC], f32, tag="gamma", name="gamma_col")
        row_to_col(g_ps, gamma_col, "g")

        # ---- query side ----
        qpT = [big.tile([P, DC, IB], fp16, tag=f"qpT{i}", name=f"qpT{i}")
               for i in range(NB)]
        qstage = {}

        def q_dma(ibk):
            tiles = []
            for cc in range(DC):
                t = qld.tile([P, D], f32, tag="qld", name=f"q_{ibk}_{cc}")
                nc.sync.dma_start(t, query[ibk * IB + cc * P: ibk * IB + (cc + 1) * P, :])
                tiles.append(t)
            qstage[ibk] = tiles

        def q_prep(ibk):
            qinT = smallp.tile([P, DC, IB], fp16, tag="qinT", name=f"qinT{ibk}")
            for cc in range(DC):
                qc = cst.tile([P, D], fp16, tag="qcast", name=f"qc_{ibk}_{cc}")
                nc.vector.tensor_copy(out=qc, in_=qstage[ibk][cc])
                psv = ps_m.tile([P, D], fp16, tag="ps_m", name=f"qT_{ibk}_{cc}")
                for j in range(DC):
                    nc.tensor.transpose(psv[:, j * P:(j + 1) * P],
                                        qc[:, j * P:(j + 1) * P], ident_h)
                nc.scalar.activation(qinT[:, :, cc * P:(cc + 1) * P],
                                     psv.rearrange("p (c q) -> p c q", c=DC),
                                     AF.Copy)
            for ec in range(DC):
                ps = ps_m.tile([P, IB], f32, tag="ps_m", name=f"qp_{ibk}_{ec}")
                for dinc in range(DC):
                    nc.tensor.matmul(ps, WQK[:, dinc, ec * P:(ec + 1) * P],
                                     qinT[:, dinc, :],
                                     start=(dinc == 0), stop=(dinc == DC - 1))
                nc.scalar.activation(qpT[ibk][:, ec, :], ps, AF.Identity,
                                     bias=gamma_col[:, ec:ec + 1])

        q_dma(0)
        q_prep(0)

        # ---- target tiles (filled just-in-time during ib 0) ----
        tgtT = [big.tile([P, DC, IB], fp16, tag=f"tgtT{i}", name=f"tgtT{i}")
                for i in range(KB)]
        tgt_h = [big.tile([P, D], fp16, tag=f"tgh{i}", name=f"tgh{i}")
                 for i in range(SC)]

        def t_dma(g):
            tiles = []
            for cc in range(DC):
                t = ld.tile([P, D], f32, tag="ld", name=f"t_{g}_{cc}")
                nc.sync.dma_start(t, target[g * IB + cc * P: g * IB + (cc + 1) * P, :])
                tiles.append(t)
            return tiles

        tstage = {0: t_dma(0)}

        def t_prep(g):
            for cc in range(DC):
                sccc = g * DC + cc
                nc.vector.tensor_copy(out=tgt_h[sccc], in_=tstage[g][cc])
                psv = ps_m.tile([P, D], fp16, tag="ps_m", name=f"tT_{g}_{cc}")
                for j in range(DC):
                    nc.tensor.transpose(psv[:, j * P:(j + 1) * P],
                                        tgt_h[sccc][:, j * P:(j + 1) * P], ident_h)
                nc.scalar.activation(tgtT[g][:, :, cc * P:(cc + 1) * P],
                                     psv.rearrange("p (c q) -> p c q", c=DC),
                                     AF.Copy)
            del tstage[g]

        # remaining DMAs in consumption order on the sync queue
        wv_h = stage_weight("wv")
        wo_h = stage_weight("wo")
        for g in range(1, 3):
            tstage[g] = t_dma(g)
        q_dma(1)
        for g in range(3, 5):
            tstage[g] = t_dma(g)
        q_dma(2)
        for g in range(5, KB):
            tstage[g] = t_dma(g)
        q_dma(3)

        WVO = const.tile([P, DC, D], bf16, tag="WVO", name="WVO")
        b_out_rep = const.tile([P, D], f32, tag="b_out", name="b_out_rep")

        def vo_prep():
            wvT = const.tile([P, DC, D], fp16, tag="wvT", name="wvT")
            transpose_into(wvT, wv_h, "wv")
            for ec in range(DC):
                ps = ps_m.tile([P, D], f32, tag="ps_m", name=f"wvo_{ec}")
                for dc in range(DC):
                    nc.tensor.matmul(ps, wvT[:, dc, ec * P:(ec + 1) * P],
                                     wo_h[:, dc, :], start=(dc == 0), stop=(dc == DC - 1))
                nc.scalar.activation(WVO[:, ec, :], ps, AF.Copy)
            bp = ps_m.tile([1, D], f32, tag="ps_m", name="bvo_ps")
            for dc in range(DC):
                nc.tensor.matmul(bp, b_col_h["bv"][:, dc:dc + 1], wo_h[:, dc, :],
                                 start=(dc == 0), stop=(dc == DC - 1))
            br = const.tile([1, D], f32, tag="b_row", name="b_out_row")
            nc.vector.tensor_tensor(br, bp, bo_row, OP.add)
            brh = const.tile([1, D], fp16, tag="b_rowh", name="b_out_row_h")
            nc.vector.tensor_copy(out=brh, in_=br)
            bp2 = ps_m.tile([P, D], f32, tag="ps_m", name="brep_ps")
            nc.tensor.matmul(bp2, ones_row_h, brh, start=True, stop=True)
            nc.scalar.activation(b_out_rep, bp2, AF.Copy)

        # ---- attention ----
        for ib in range(NB):
            ctx_ps = [ps_ctx.tile([P, IB], f32, tag="ps_ctx", name=f"ctx_{ib}_{d}")
                      for d in range(DC)]
            acc = cst.tile([P, IB], f32, tag="acc", name=f"acc_{ib}")

            for scc in range(SC):
                g, sl = divmod(scc, IB // P)
                if ib == 0 and sl == 0:
                    t_prep(g)
                pt_ps = ps_m.tile([P, IB], f32, tag="ps_m", name=f"pt_{ib}_{scc}")
                for ec in range(DC):
                    nc.tensor.matmul(pt_ps, tgtT[g][:, ec, sl * P:(sl + 1) * P],
                                     qpT[ib][:, ec, :],
                                     start=(ec == 0), stop=(ec == DC - 1))
                pt_exp = ptp.tile([P, IB], fp16, tag="pt_exp", name=f"pte_{ib}_{scc}")
                nc.scalar.activation(pt_exp, pt_ps, AF.Exp, bias=negc_col)
                if scc == 0:
                    nc.vector.tensor_copy(out=acc, in_=pt_exp)
                else:
                    nc.vector.tensor_tensor(acc, acc, pt_exp, OP.add)
                for ec in range(DC):
                    nc.tensor.matmul(ctx_ps[ec], tgt_h[scc][:, ec * P:(ec + 1) * P],
                                     pt_exp, start=(scc == 0), stop=(scc == SC - 1))
                if ib == 0 and scc == 6:
                    vo_prep()
                if ib < NB - 1 and scc == 20:
                    q_prep(ib + 1)

            # softmax denominators
            rs_ps = ps_m.tile([1, IB], f32, tag="ps_m", name=f"rs_{ib}")
            nc.tensor.matmul(rs_ps, ones_col, acc, start=True, stop=True)
            rsum_col = cst.tile([P, DC], f32, tag="rsc", name=f"rsc_{ib}")
            row_to_col(rs_ps, rsum_col, f"rs{ib}")
            rc_col = cst.tile([P, DC], f32, tag="rcc", name=f"rc_{ib}")
            nc.vector.reciprocal(rc_col, rsum_col)

            ctxT = ctxp.tile([P, DC, IB], bf16, tag="ctxT", name=f"ctxT_{ib}")
            for ec in range(DC):
                nc.scalar.activation(ctxT[:, ec, :], ctx_ps[ec], AF.Copy)

            for qc in range(DC):
                op_ps = ps_m.tile([P, D], f32, tag="ps_m", name=f"op_{ib}_{qc}")
                for ec in range(DC):
                    nc.tensor.matmul(op_ps, ctxT[:, ec, qc * P:(qc + 1) * P],
                                     WVO[:, ec, :], start=(ec == 0), stop=(ec == DC - 1))
                ot_s = outp.tile([P, D], f32, tag="out_s", name=f"ots_{ib}_{qc}")
                nc.scalar.activation(ot_s, op_ps, AF.Copy,
                                     scale=rc_col[:, qc:qc + 1])
                ot = outp.tile([P, D], f32, tag="out_t", name=f"ot_{ib}_{qc}")
                nc.vector.tensor_tensor(ot, ot_s, b_out_rep, OP.add)
                nc.gpsimd.dma_start(
                    out_dram[ib * IB + qc * P: ib * IB + (qc + 1) * P, :], ot)

    nc.compile()
    return nc


def _get_nc():
    if "nc" not in _CACHED:
        _CACHED["nc"] = _build_program()
    return _CACHED["nc"]


def _make_in_maps(query, target, wq, bq, wk, bk, wv, bv, wo, bo):
    query = np.asarray(query, dtype=np.float32)
    target = np.asarray(target, dtype=np.float32)
    consts = {
        "wq": np.asarray(wq, np.float32), "bq": np.asarray(bq, np.float32),
        "wk": np.asarray(wk, np.float32),
        "wv": np.asarray(wv, np.float32), "bv": np.asarray(bv, np.float32),
        "wo": np.asarray(wo, np.float32), "bo": np.asarray(bo, np.float32),
    }
    in_maps = []
    for core in range(8):
        b, h = divmod(core, 2)
        in_maps.append({
            "query": np.ascontiguousarray(query[b, h * LQH:(h + 1) * LQH]),
            # faithful to the torch reshape: raw reinterpret of [512, 4096]
            "target": np.ascontiguousarray(target[b]).reshape(S, D),
            **consts,
        })
    return in_maps


def kernel(query, target, wq, bq, wk, bk, wv, bv, wo, bo):
    from concourse import bass_utils
    nc = _get_nc()
    in_maps = _make_in_maps(query, target, wq, bq, wk, bk, wv, bv, wo, bo)
    res = bass_utils.run_bass_kernel_spmd(nc, in_maps, core_ids=list(range(8)))
    out = np.empty((B, LQ, D), np.float32)
    for core in range(8):
        b, h = divmod(core, 2)
        out[b, h * LQH:(h + 1) * LQH] = res.results[core]["out"]
    return out
